# revision 2
# baseline (speedup 1.0000x reference)
"""GCN encoder (2x GCNConv + mu/logstd heads) on 8 Trainium2 NeuronCores.

v2 — scatter-add formulation (src-sharded):
  Per-edge random access is done with indirect scatter-ADD DMAs
  (~58us/instruction vs ~100us for indirect gathers on this runtime);
  the aggregation happens IN DRAM via the DMA's CCE add:

  per layer l:  hs = dis * (h_own @ W_l)     (PE + DVE, local slice;
                                              GCN ew = dis[src]*dis[dst])
                agg_full[pad(dst)] += hs[pad(src)]  (indirect scatter-add,
                                              one inst per out-deg-class col)
                ReduceScatter(add) over 8 cores -> agg_own
                nxt = act(dis*(agg_own + hs_own) + b)  (self loop = dis^2 h')

  Within one scatter instruction the 128 partition targets must be
  distinct rows (in-DRAM RMW add loses colliding updates) — the host
  schedule assigns edges to columns collision-free.  Cross-instruction
  accumulation is safe (verified on HW).  mu and logstd share layer 3
  (W_cat = [W_mu | W_ls], 2*32 = 64 = ch_hid, so every layer moves 64ch).
"""

import numpy as np

from concourse import bass, tile, bacc
from concourse import bass_utils

mybir = bass.mybir

LEAKY_SLOPE = 0.01
N_CORES = 8
P = 128
CLASS_MIN = 2048


class Plan:
    pass


def build_plan(edge_index, n_nodes):
    src = np.asarray(edge_index[0], dtype=np.int64)
    dst = np.asarray(edge_index[1], dtype=np.int64)

    deg_in = np.bincount(dst, minlength=n_nodes).astype(np.int64) + 1
    dis = (1.0 / np.sqrt(deg_in.astype(np.float64))).astype(np.float32)
    out_deg = np.bincount(src, minlength=n_nodes).astype(np.int64)

    # snake-deal nodes to cores by out-degree (balances scatter columns)
    order = np.argsort(out_deg, kind="stable")
    pattern = np.concatenate([np.arange(N_CORES), np.arange(N_CORES - 1, -1, -1)])
    core_of = np.empty(n_nodes, dtype=np.int32)
    reps = (n_nodes + 2 * N_CORES - 1) // (2 * N_CORES)
    core_of[order] = np.tile(pattern, reps)[:n_nodes]

    # out-degree classes (greedy merge)
    maxd = int(out_deg.max())
    hist = np.bincount(out_deg, minlength=maxd + 1)
    classes = []
    lo, acc = None, 0
    for d in range(maxd + 1):
        if hist[d] == 0 and lo is None:
            continue
        if lo is None:
            lo = d
        acc += int(hist[d])
        if acc >= CLASS_MIN or d == 0:
            classes.append((lo, d))
            lo, acc = None, 0
    if lo is not None:
        classes.append((lo, maxd))
    ncls = len(classes)
    cod = np.zeros(maxd + 1, np.int32)
    for ci, (a, b) in enumerate(classes):
        cod[a:b + 1] = ci
    cls_of = cod[out_deg]
    Ks = np.array([b for (a, b) in classes], dtype=np.int64)

    counts = np.zeros((N_CORES, ncls), np.int64)
    for c in range(N_CORES):
        counts[c] = np.bincount(cls_of[core_of == c], minlength=ncls)
    n_rows = np.ceil(counts.max(axis=0) / P).astype(np.int64)
    row_base = np.concatenate([[0], np.cumsum(n_rows)])[:-1]
    Np = int(n_rows.sum())
    Rc = P * Np
    Rtot = N_CORES * Rc
    DUMP = Rtot  # scatter target for padding slots

    # padded id: core c, class ci, member m -> p = m % P, jj = row_base + m//P
    pad_id = np.full(n_nodes, -1, dtype=np.int64)
    node_at = np.full((N_CORES, P, Np), -1, dtype=np.int64)
    for c in range(N_CORES):
        cm = core_of == c
        for ci in range(ncls):
            mem = np.where(cm & (cls_of == ci))[0]
            if mem.shape[0] == 0:
                continue
            m_idx = np.arange(mem.shape[0])
            pp = m_idx % P
            jj = row_base[ci] + m_idx // P
            pad_id[mem] = c * Rc + pp * Np + jj
            node_at[c, pp, jj] = mem

    # global column schedule: (jj, k) for every class/row/slot — identical on
    # all cores (SPMD); only the idx tables differ per core.
    sched = []
    for ci in range(ncls):
        K = int(Ks[ci])
        for q in range(int(n_rows[ci])):
            jj = int(row_base[ci] + q)
            for k in range(K):
                sched.append(jj)
    qof = np.asarray(sched, dtype=np.int32)
    S = qof.shape[0]

    # per-core idx tables with within-column distinct targets
    e_by_src = np.argsort(src, kind="stable")
    src_ptr = np.concatenate([[0], np.cumsum(np.bincount(src, minlength=n_nodes))])
    dst_pad_sorted = pad_id[dst[e_by_src]]

    idx_arr = np.full((N_CORES, P, S), DUMP, dtype=np.int32)
    col_of_block = {}
    o = 0
    blocks = []
    for ci in range(ncls):
        K = int(Ks[ci])
        for q in range(int(n_rows[ci])):
            jj = int(row_base[ci] + q)
            blocks.append((jj, o, K))
            o += K
    assert o == S

    for c in range(N_CORES):
        for (jj, col0, K) in blocks:
            if K == 0:
                continue
            block = np.full((P, K), DUMP, dtype=np.int64)
            for p in range(P):
                n = node_at[c, p, jj]
                if n < 0:
                    continue
                a, b = int(src_ptr[n]), int(src_ptr[n + 1])
                t = dst_pad_sorted[a:b]
                block[p, :t.shape[0]] = t
            # collision resolution: per column, all non-DUMP targets distinct
            colsets = [set() for _ in range(K)]
            for k in range(K):
                for p in range(P):
                    v = int(block[p, k])
                    if v == DUMP:
                        continue
                    if v not in colsets[k]:
                        colsets[k].add(v)
                        continue
                    done = False
                    for k2 in range(K):
                        if k2 == k:
                            continue
                        v2 = int(block[p, k2])
                        if v2 != DUMP and v2 in colsets[k]:
                            continue
                        if v in colsets[k2]:
                            continue
                        # swap
                        block[p, k], block[p, k2] = v2, v
                        colsets[k2].discard(v2) if False else None
                        if k2 < k:
                            # maintain finalized set for k2
                            if v2 != DUMP:
                                colsets[k2].discard(v2)
                            colsets[k2].add(v)
                        if v2 != DUMP:
                            colsets[k].add(v2)
                        done = True
                        break
                    assert done, "unresolvable scatter collision"
            idx_arr[c, :, col0:col0 + K] = block

    dis_pad = np.zeros(Rtot, np.float32)
    dis_pad[pad_id] = dis

    plan = Plan()
    plan.n_nodes, plan.Np, plan.Rc, plan.Rtot, plan.S = n_nodes, Np, Rc, Rtot, S
    plan.idx_arr, plan.qof = idx_arr, qof
    plan.pad_id, plan.dis_pad, plan.core_of = pad_id, dis_pad, core_of
    return plan


def build_nc(plan, ch_in=128, ch_hid=64, ch_out=32, reps=1):
    Np, Rc, Rtot, S = plan.Np, plan.Rc, plan.Rtot, plan.S
    qof = plan.qof
    f32 = mybir.dt.float32
    ch3 = 2 * ch_out
    assert ch3 == ch_hid  # every layer aggregates ch_hid channels

    nc = bacc.Bacc("TRN2", target_bir_lowering=False, debug=False,
                   num_devices=N_CORES, num_swdge_queues=1)

    x_e = nc.dram_tensor("x_own", [Rc, ch_in], f32, kind="ExternalInput")
    idx_e = nc.dram_tensor("idx", [P, S], mybir.dt.int32, kind="ExternalInput")
    dis_e = nc.dram_tensor("disr", [P, Np], f32, kind="ExternalInput")
    w1_e = nc.dram_tensor("W1", [ch_in, ch_hid], f32, kind="ExternalInput")
    w2_e = nc.dram_tensor("W2", [ch_hid, ch_hid], f32, kind="ExternalInput")
    w3_e = nc.dram_tensor("W3", [ch_hid, ch3], f32, kind="ExternalInput")
    b1_e = nc.dram_tensor("b1r", [P, ch_hid], f32, kind="ExternalInput")
    b2_e = nc.dram_tensor("b2r", [P, ch_hid], f32, kind="ExternalInput")
    b3_e = nc.dram_tensor("b3r", [P, ch3], f32, kind="ExternalInput")
    id_e = nc.dram_tensor("ident", [P, P], f32, kind="ExternalInput")
    mu_e = nc.dram_tensor("mu", [Rc, ch_out], f32, kind="ExternalOutput")
    ls_e = nc.dram_tensor("ls", [Rc, ch_out], f32, kind="ExternalOutput")

    agg_full = nc.dram_tensor("agg_full", [Rtot + P, ch_hid], f32)
    agg_own = nc.dram_tensor("agg_own", [Rc, ch_hid], f32)

    ZCH = 2048 // ch_hid          # zero-DMA rows per partition
    n_zrows = Rtot // P + 1       # node rows per partition (+1 dump block)
    n_zdma = (n_zrows + ZCH - 1) // ZCH

    with tile.TileContext(nc) as tc:
        with tc.tile_pool(name="persist", bufs=1) as persist, \
             tc.tile_pool(name="mbuf", bufs=3) as mbuf, \
             tc.tile_pool(name="psum", bufs=4, space="PSUM") as psum:

            idx_sb = persist.tile([P, S], mybir.dt.int32)
            nc.sync.dma_start(out=idx_sb[:], in_=idx_e[:])
            dis_sb = persist.tile([P, Np], f32)
            nc.sync.dma_start(out=dis_sb[:], in_=dis_e[:])
            w1_sb = persist.tile([ch_in, ch_hid], f32)
            w2_sb = persist.tile([ch_hid, ch_hid], f32)
            w3_sb = persist.tile([ch_hid, ch3], f32)
            b1_sb = persist.tile([P, ch_hid], f32)
            b2_sb = persist.tile([P, ch_hid], f32)
            b3_sb = persist.tile([P, ch3], f32)
            ident = persist.tile([P, P], f32)
            for sb, e in ((w1_sb, w1_e), (w2_sb, w2_e), (w3_sb, w3_e),
                          (b1_sb, b1_e), (b2_sb, b2_e), (b3_sb, b3_e),
                          (ident, id_e)):
                nc.sync.dma_start(out=sb[:], in_=e[:])

            cur = persist.tile([P, Np, ch_in], f32, tag="cur")
            hs = persist.tile([P, Np, ch_hid], f32, tag="hs")
            aggs = persist.tile([P, Np, ch_hid], f32, tag="aggs")
            zt = persist.tile([P, ZCH * ch_hid], f32, tag="zt")
            nc.vector.memset(zt[:], 0.0)

            def matmul_hs(ch_i, w_sb):
                """hs = dis * (cur[:, :, :ch_i] @ W)"""
                for jj in range(Np):
                    at = psum.tile([ch_i, P], f32, tag="atp")
                    nc.tensor.transpose(out=at[:], in_=cur[:, jj, :ch_i],
                                        identity=ident[:])
                    at_sb = mbuf.tile([ch_i, P], f32, tag="atsb")
                    nc.scalar.copy(out=at_sb[:], in_=at[:])
                    ot = psum.tile([P, ch_hid], f32, tag="otp")
                    nc.tensor.matmul(out=ot[:], lhsT=at_sb[:],
                                     rhs=w_sb[:], start=True, stop=True)
                    nc.vector.tensor_tensor(
                        out=hs[:, jj, :], in0=ot[:],
                        in1=dis_sb[:, jj:jj + 1].broadcast_to([P, ch_hid]),
                        op=mybir.AluOpType.mult)

            for _rep in range(reps):
                nc.sync.dma_start(
                    out=cur[:], in_=x_e[:].rearrange("(p n) c -> p n c", p=P))

                for (ch_i, w_sb, b_sb, act) in (
                        (ch_in, w1_sb, b1_sb, True),
                        (ch_hid, w2_sb, b2_sb, True),
                        (ch_hid, w3_sb, b3_sb, False)):
                    matmul_hs(ch_i, w_sb)

                    # zero the partial-agg table
                    for z in range(n_zdma):
                        r0 = z * ZCH
                        r1 = min(n_zrows, r0 + ZCH)
                        nc.sync.dma_start(
                            out=agg_full[:].rearrange(
                                "(p n) c -> p n c", p=P)[:, r0:r1, :],
                            in_=zt[:, :(r1 - r0) * ch_hid].rearrange(
                                "p (n c) -> p n c", c=ch_hid))

                    # scatter-add every out-edge slot
                    for col in range(S):
                        q = int(qof[col])
                        nc.gpsimd.indirect_dma_start(
                            out=agg_full[:], in_=hs[:, q, :],
                            in_offset=None,
                            out_offset=bass.IndirectOffsetOnAxis(
                                ap=idx_sb[:, col:col + 1], axis=0),
                            compute_op=mybir.AluOpType.add)

                    # combine partials across cores
                    nc.gpsimd.collective_compute(
                        "ReduceScatter", mybir.AluOpType.add,
                        replica_groups=[list(range(N_CORES))],
                        ins=[agg_full[0:Rtot, :].opt()],
                        outs=[agg_own.ap().opt()])

                    nc.sync.dma_start(
                        out=aggs[:],
                        in_=agg_own[:].rearrange("(p n) c -> p n c", p=P))

                    # nxt = act(dis * (agg_own + hs) + b)
                    for jj in range(Np):
                        t = cur[:, jj, :ch_hid]
                        nc.vector.tensor_tensor(
                            out=t, in0=aggs[:, jj, :], in1=hs[:, jj, :],
                            op=mybir.AluOpType.add)
                        nc.vector.tensor_tensor(
                            out=t, in0=t,
                            in1=dis_sb[:, jj:jj + 1].broadcast_to([P, ch_hid]),
                            op=mybir.AluOpType.mult)
                        nc.vector.tensor_tensor(
                            out=t, in0=t, in1=b_sb[:],
                            op=mybir.AluOpType.add)
                        if act:
                            nc.vector.scalar_tensor_tensor(
                                out=t, in0=t, scalar=LEAKY_SLOPE, in1=t,
                                op0=mybir.AluOpType.mult,
                                op1=mybir.AluOpType.max)

                nc.sync.dma_start(
                    out=mu_e[:].rearrange("(p n) c -> p n c", p=P),
                    in_=cur[:, :, :ch_out])
                nc.sync.dma_start(
                    out=ls_e[:].rearrange("(p n) c -> p n c", p=P),
                    in_=cur[:, :, ch_out:2 * ch_out])

    nc.compile()
    return nc


def make_in_maps(plan, x, W1, b1, W2, b2, W_mu, b_mu, W_ls, b_ls):
    n_nodes, ch_in = x.shape
    Rc, Np = plan.Rc, plan.Np
    x_pad = np.zeros((plan.Rtot, ch_in), dtype=np.float32)
    x_pad[plan.pad_id] = np.asarray(x, np.float32)
    w3 = np.concatenate([np.asarray(W_mu), np.asarray(W_ls)], axis=1) \
        .astype(np.float32)
    b3 = np.concatenate([np.asarray(b_mu), np.asarray(b_ls)]).astype(np.float32)
    ident = np.eye(P, dtype=np.float32)
    in_maps = []
    for c in range(N_CORES):
        in_maps.append({
            "x_own": x_pad[c * Rc:(c + 1) * Rc],
            "idx": plan.idx_arr[c],
            "disr": plan.dis_pad[c * Rc:(c + 1) * Rc].reshape(P, Np),
            "W1": np.asarray(W1, np.float32),
            "W2": np.asarray(W2, np.float32),
            "W3": w3,
            "b1r": np.tile(np.asarray(b1, np.float32)[None, :], (P, 1)),
            "b2r": np.tile(np.asarray(b2, np.float32)[None, :], (P, 1)),
            "b3r": np.tile(b3[None, :], (P, 1)),
            "ident": ident,
        })
    return in_maps


def unshard(plan, results, ch_out=32):
    mu = np.zeros((plan.n_nodes, ch_out), dtype=np.float32)
    ls = np.zeros((plan.n_nodes, ch_out), dtype=np.float32)
    Rc = plan.Rc
    for c in range(N_CORES):
        nodes = np.where((plan.pad_id >= c * Rc) & (plan.pad_id < (c + 1) * Rc))[0]
        rows = plan.pad_id[nodes] - c * Rc
        mu[nodes] = results[c]["mu"][rows]
        ls[nodes] = results[c]["ls"][rows]
    return mu, ls


def run(x, edge_index, W1, b1, W2, b2, W_mu, b_mu, W_ls, b_ls, sim=False):
    x = np.asarray(x, dtype=np.float32)
    plan = build_plan(edge_index, x.shape[0])
    nc = build_nc(plan, ch_in=x.shape[1], ch_hid=np.asarray(W1).shape[1],
                  ch_out=np.asarray(W_mu).shape[1])
    in_maps = make_in_maps(plan, x, W1, b1, W2, b2, W_mu, b_mu, W_ls, b_ls)
    if sim:
        from concourse.bass_interp import MultiCoreSim
        msim = MultiCoreSim(nc, num_cores=N_CORES)
        for c in range(N_CORES):
            for k, v in in_maps[c].items():
                msim.cores[c].tensor(k)[:] = v
        msim.simulate()
        results = [{"mu": np.array(msim.cores[c].tensor("mu")),
                    "ls": np.array(msim.cores[c].tensor("ls"))}
                   for c in range(N_CORES)]
    else:
        res = bass_utils.run_bass_kernel_spmd(
            nc, in_maps, core_ids=list(range(N_CORES)))
        results = res.results
    return unshard(plan, results)


def kernel(x, edge_index, W1, b1, W2, b2, W_mu, b_mu, W_ls, b_ls):
    return run(x, edge_index, W1, b1, W2, b2, W_mu, b_mu, W_ls, b_ls)
